# revision 47
# baseline (speedup 1.0000x reference)
# Trainium2 Bass kernel for nn_PointGridExtractor (submanifold sparse 3D CNN).
#
# The voxel grid is astronomically sparse (2048 active sites in up to 2.4e11
# cells), so each submanifold conv is a center-tap GEMM plus a small set of
# real neighbor pairs (0/0/34/128/838 pairs for stages 0..4 on this input).
# All index structure depends only on `coors` and is computed host-side; the
# device does dense GEMM/BN/ReLU work:
#   - activations in SBUF as [channel_partitions, voxel_column] (float32r)
#   - center conv: PE matmuls, weights stationary
#   - neighbor pairs: gather src columns (GPSIMD ap_gather for small stages;
#     PE transpose of the source window + selection matmul for stage 4),
#     per-offset transposed GEMMs (bf16 weights halve the dominant ~40MB
#     pair-weight stream), one-hot scatter matmul accumulates into the main
#     conv PSUM
#   - pool3's 18 max-merges become virtual "delta" pairs
#     (delta = relu(extra - survivor) feeds the next conv); pool4 merges and
#     the final dense scatter run on the host
#   - BN stats via bn_stats/bn_aggr over a contiguous active-column prefix
#     (column order puts dead columns last); fused relu(scale*x+shift) on ACT
# The 8 cores run the identical program: at this size every cross-core
# exchange (collectives measured at 25-50us each) costs more than local
# recompute, so full replication is the fastest distribution.
import numpy as np
import ml_dtypes

N_VOX = 2048
DELTA = 32              # columns [0, DELTA): merge-delta slots + zero pad
W_COLS = DELTA + N_VOX
ZERO_COL = DELTA - 1    # always-zero column (gather pad target)
DIMS = [(216, 7992, 7056), (72, 2664, 2352), (24, 888, 784), (8, 444, 392), (4, 222, 196), (2, 111, 98)]
KS = [7, 7, 7, 7, 5, 5, 3, 3, 3, 3]
POOLS = [(3, 3, 3), (3, 3, 3), (3, 2, 2), (2, 2, 2), (2, 2, 2)]
CIN = [1, 64, 64, 128, 128, 256, 256, 384, 384, 512]
COUT = [64, 64, 128, 128, 256, 256, 384, 384, 512, 512]
BIG = np.int64(2 ** 62)
EPS = 1e-4
BATCH = 2
# pair-GEMM dtype per stage: fp8e4 weights/activations (with per-wid
# power-of-2 scales folded into the yt copy) vs bf16.  fp8 measured ~2.6%
# acc error per stage-4 layer (pair sums are random-walk sized; quant noise
# does not average out) — too lossy for the 2e-2 budget, so bf16.
FP8S = {2: False, 3: False, 4: False}


def _encode(c, b, dims):
    D, H, W = dims
    return ((b * np.int64(D) + c[:, 0]) * H + c[:, 1]) * W + c[:, 2]


def _offsets(k):
    r = k // 2
    g = np.arange(-r, r + 1)
    return np.stack(np.meshgrid(g, g, g, indexing='ij'), -1).reshape(-1, 3).astype(np.int64)


# --------------------------------------------------------------------------
# host index pipeline
# --------------------------------------------------------------------------

def host_prep(coors):
    coords = coors[:, 1:4].astype(np.int64)
    bidx = coors[:, 0].astype(np.int64)
    N = coords.shape[0]
    assert N == N_VOX
    active = np.ones(N, bool)
    stage_pairs = []   # s -> (dst, src, noncenter_off_index)
    stage_active = []
    pool_groups = []
    for s in range(5):
        stage_active.append(active.copy())
        dims = DIMS[s]
        k = KS[2 * s]
        keys = np.where(active, _encode(coords, bidx, dims), BIG)
        order = np.argsort(keys, kind='stable')
        skeys = keys[order]
        offs = _offsets(k)
        ctr = len(offs) // 2
        dsts, srcs, oids = [], [], []
        act_idx = np.where(active)[0]
        noff = 0
        for oi, off in enumerate(offs):
            if oi == ctr:
                continue
            nc_ = coords[act_idx] + off[None, :]
            ok = ((nc_ >= 0) & (nc_ < np.array(dims)[None, :])).all(1)
            nkey = np.where(ok, _encode(nc_, bidx[act_idx], dims), np.int64(-1))
            pos = np.clip(np.searchsorted(skeys, nkey), 0, N - 1)
            hit = ok & (skeys[pos] == nkey)
            if hit.any():
                dsts.append(act_idx[hit])
                srcs.append(order[pos[hit]])
                oids.append(np.full(int(hit.sum()), noff))
            noff += 1
        if dsts:
            stage_pairs.append((np.concatenate(dsts), np.concatenate(srcs),
                                np.concatenate(oids)))
        else:
            stage_pairs.append((np.zeros(0, np.int64),) * 3)
        stride = np.array(POOLS[s], np.int64)
        odims = DIMS[s + 1]
        pc = coords // stride[None, :]
        pkeys = np.where(active, _encode(pc, bidx, odims), BIG)
        groups = {}
        for v in act_idx:
            groups.setdefault(int(pkeys[v]), []).append(int(v))
        new_active = np.zeros(N, bool)
        gmulti = []
        for key, members in groups.items():
            surv = members[0]
            new_active[surv] = True
            D, H, W = odims
            w = key % W; t = key // W
            h = t % H; t = t // H
            d = t % D; b = t // D
            coords[surv] = (d, h, w)
            bidx[surv] = b
            if len(members) > 1:
                gmulti.append(members)
        pool_groups.append(gmulti)
        active = new_active
    return dict(stage_pairs=stage_pairs, stage_active=stage_active,
                pool_groups=pool_groups, final_active=active,
                final_coords=coords, final_bidx=bidx)


def _pad16(n):
    return max(16, ((n + 15) // 16) * 16)


def build_structure(coors):
    prep = host_prep(coors)
    sp = prep['stage_pairs']
    pg3 = prep['pool_groups'][3]
    n_mrg = len(pg3)
    assert n_mrg <= DELTA - 1, n_mrg
    assert all(len(m) == 2 for m in pg3), "pool3 groups expected size 2"
    dead3 = np.array(sorted(e for m in pg3 for e in m[1:]), np.int64)
    surv3 = {m[0]: g for g, m in enumerate(pg3)}

    # ---- column order: [0,DELTA) deltas | srcs4 | dst4-only | rest | dead3
    src4 = np.unique(sp[4][1]) if len(sp[4][0]) else np.zeros(0, np.int64)
    dst4 = np.unique(np.concatenate([sp[4][0], np.fromiter(surv3.keys(), np.int64, len(surv3))])) if (len(sp[4][0]) or surv3) else np.zeros(0, np.int64)
    dead_set = set(dead3.tolist())
    assert not (set(src4.tolist()) | set(dst4.tolist())) & dead_set
    ordered, seen = [], set(dead_set)
    for v in src4:
        if int(v) not in seen:
            ordered.append(int(v)); seen.add(int(v))
    n_src4 = len(ordered)
    for v in dst4:
        if int(v) not in seen:
            ordered.append(int(v)); seen.add(int(v))
    for v in range(N_VOX):
        if v not in seen:
            ordered.append(v); seen.add(v)
    ordered.extend(int(x) for x in dead3)
    assert len(ordered) == N_VOX
    col = np.zeros(N_VOX, np.int64)
    for pos, v in enumerate(ordered):
        col[v] = DELTA + pos
    SRCWIN = ((DELTA + n_src4 + 127) // 128) * 128

    n_act = [int(a.sum()) for a in prep['stage_active']]
    assert n_act[:4] == [N_VOX] * 4
    act4_cols = np.sort(col[prep['stage_active'][4]])
    assert act4_cols[-1] == DELTA + n_act[4] - 1, "active prefix broken"

    def mk_slots_small(s):
        d, sv, oi = sp[s]
        order = np.argsort(oi, kind='stable')
        d, sv, oi = d[order], sv[order], oi[order]
        wids = np.unique(oi)
        wid_of = {int(o): i for i, o in enumerate(wids)}
        slots = [(int(col[a]), int(col[b]), wid_of[int(o)]) for a, b, o in zip(d, sv, oi)]
        return slots, [int(o) for o in wids]

    slots2, act_off2 = mk_slots_small(2)
    slots3, act_off3 = mk_slots_small(3)

    d4, s4, o4 = sp[4]
    wids4 = np.unique(o4)
    wid_of4 = {int(o): i for i, o in enumerate(wids4)}
    CENTER_WID4 = len(wids4)
    raw = []
    for a, b, o in zip(d4, s4, o4):
        raw.append((int(col[a]), int(col[b]), wid_of4[int(o)]))
        if int(b) in surv3:
            raw.append((int(col[a]), surv3[int(b)], wid_of4[int(o)]))
    for sv_vox, g in surv3.items():
        raw.append((int(col[sv_vox]), g, CENTER_WID4))
    slots4 = sorted(raw, key=lambda t: t[2])
    act_off4 = [int(o) for o in wids4]

    # ---- stage-4 packed block layout (transposed pair GEMM) ----
    # Each wid gets a 32-padded block; blocks may not cross 128-group
    # boundaries and must start at 0/32/64 within a group (PE tile_position
    # constraint; 96 is not a legal base).  Sizes <=64 assumed (assert).
    by_wid = {}
    for (dc, sc, w) in slots4:
        by_wid.setdefault(w, []).append((dc, sc))
    # sizes padded to 32; legal block starts within a group: 0/32/64, with
    # 64-blocks only at 0/64 and >=96-blocks only at 0.  Optimal grouping:
    # (32,32,64) full groups, then (64,64), then 32-triples (32 wasted).
    psz = {w: ((len(by_wid[w]) + 31) // 32) * 32 for w in by_wid}
    assert all(v <= 128 for v in psz.values())
    b96 = [w for w in by_wid if psz[w] >= 96]
    b64 = [w for w in by_wid if psz[w] == 64]
    b32 = [w for w in by_wid if psz[w] == 32]
    group_layouts = []   # list of [(wid, start, padded)]
    for w in b96:
        group_layouts.append([(w, 0, psz[w])])
    while b64 and len(b32) >= 2:
        a, b = b32.pop(), b32.pop()
        c = b64.pop()
        group_layouts.append([(a, 0, 32), (b, 32, 32), (c, 64, 64)])
    while len(b64) >= 2:
        c, d = b64.pop(), b64.pop()
        group_layouts.append([(c, 0, 64), (d, 64, 64)])
    if b64:
        g = [(b64.pop(), 0, 64)]
        if b32:
            g.append((b32.pop(), 64, 32))
        group_layouts.append(g)
    while b32:
        g = []
        for start in (0, 32, 64):
            if b32:
                g.append((b32.pop(), start, 32))
        group_layouts.append(g)
    # blocks: (wid, compact_lo, compact_hi, padded_lo).  G is built compact
    # (sel has NS4c columns); the pair GEMM writes the block at its padded
    # psum position.  Pad rows hold stale-but-finite psum garbage which the
    # zero one-hot rows null out in the scatter.
    blocks = []
    pos_slots4 = {}   # padded position -> (dst, src, wid), for one-hots
    wid_neworder = []
    cpos = 0
    for gi, gl in enumerate(group_layouts):
        for (w, start, padded) in gl:
            n = len(by_wid[w])
            blocks.append((w, cpos, cpos + n, gi * 128 + start))
            wid_neworder.append(w)
            for i, (dc, sc) in enumerate(by_wid[w]):
                pos_slots4[gi * 128 + start + i] = (dc, sc, w)
            cpos += n
    NG4 = len(group_layouts)
    NS4c = _pad16(cpos)
    # compact slot list for sel
    compact_slots4 = []
    for (w, clo, chi, plo) in blocks:
        compact_slots4.extend((dc, sc, w) for (dc, sc) in by_wid[w])
    widmap = {w: i for i, w in enumerate(wid_neworder)}
    blocks = [(widmap[w], clo, chi, plo) for (w, clo, chi, plo) in blocks]
    pairlist4 = [(-1 if w == CENTER_WID4 else act_off4[w]) for w in wid_neworder]

    mrg_s = np.full(DELTA, ZERO_COL, np.int64)
    mrg_e = np.full(DELTA, ZERO_COL, np.int64)
    for svx, g in surv3.items():
        mrg_s[g] = col[svx]
    for m in pg3:
        mrg_e[surv3[m[0]]] = col[m[1]]

    st = dict(prep=prep, col=col, SRCWIN=SRCWIN, n_act=n_act,
              slots=[None, None, slots2, slots3, slots4],
              act_off=[None, None, act_off2, act_off3, act_off4],
              center_wid4=CENTER_WID4, mrg_s=mrg_s, mrg_e=mrg_e, n_mrg=n_mrg,
              blocks4=blocks, pos_slots4=pos_slots4, pairlist4=pairlist4,
              compact_slots4=compact_slots4, NS4c=NS4c,
              NG4=NG4, NS4=NG4 * 128)

    # one-hot scatter map + dst spans: per stage, (group, chunk) -> [jmin, jend)
    def mk_ohmap(items, tag):
        hits = {}
        for j, (dc, sc, w) in items:
            key = (j // 128, (dc - DELTA) // 512)
            rel = dc - DELTA - key[1] * 512
            lo, hi = hits.get(key, (512, 0))
            hits[key] = (min(lo, rel), max(hi, rel + 1))
        st[f'ohmap{tag}'] = sorted(hits.keys())
        st[f'ohspan{tag}'] = hits
    mk_ohmap(list(enumerate(slots2)), 2)
    mk_ohmap(list(enumerate(slots3)), 3)
    mk_ohmap(list(pos_slots4.items()), 4)
    return st


def slot_ranges(slots):
    """Contiguous (wid, lo, hi) ranges split at 128-group boundaries."""
    n = len(slots)
    ranges = []
    i = 0
    while i < n:
        j = i
        w = slots[i][2]
        while j < n and slots[j][2] == w:
            j += 1
        lo = i
        while lo < j:
            hi = min(j, ((lo // 128) + 1) * 128)
            ranges.append((w, lo, hi))
            lo = hi
        i = j
    return ranges


# --------------------------------------------------------------------------
# device input packing
# --------------------------------------------------------------------------

def _wrap_idx(idxs, pad_to):
    out = np.zeros((128, pad_to // 16), np.int16)
    for j in range(pad_to):
        v = idxs[j] if j < len(idxs) else ZERO_COL
        out[np.arange(8) * 16 + (j % 16), j // 16] = v
    return out


def pack_structure(st, inputs):
    col = st['col']
    arrs = {}
    wid_scale = {}
    vf = np.asarray(inputs['voxel_features'], np.float32)
    f0 = np.zeros((1, W_COLS), np.float32)
    f0[0, col] = vf[:, 0]
    arrs['f0'] = f0
    arrs['identb'] = np.eye(128, dtype=ml_dtypes.bfloat16)
    for li in range(10):
        W = np.asarray(inputs[f'conv_w_{li}'], np.float32)
        k3 = KS[li] ** 3
        ctr = k3 // 2
        arrs[f'wc{li}'] = np.ascontiguousarray(W[ctr])
        g = np.asarray(inputs[f'bn_g_{li}'], np.float32)
        b = np.asarray(inputs[f'bn_b_{li}'], np.float32)
        cout_t = (COUT[li] + 127) // 128
        gb = np.zeros((cout_t, 128, 2), np.float32)
        gb[:, :, 0].reshape(-1)[:COUT[li]] = g
        gb[:, :, 1].reshape(-1)[:COUT[li]] = b
        arrs[f'gb{li}'] = gb
        s = li // 2
        if s >= 2:
            offs_nc = [oi for oi in range(k3) if oi != ctr]
            if s < 4:
                act = st['act_off'][s]
                mats = [W[offs_nc[oi]] for oi in act]
            else:
                mats = [(W[ctr] if oi < 0 else W[offs_nc[oi]]) for oi in st['pairlist4']]
            Wp = np.stack(mats)
            nwid, Ci, Co = Wp.shape
            kt = Ci // 128
            # HBM layout [kt, 128, nwid*Co]: per-partition lines are contiguous
            # nwid*Co runs so the wpt DMA moves large descriptors
            if FP8S[s]:
                aw = np.abs(Wp).reshape(nwid, -1).max(1)
                sw = np.where(aw > 0, 2.0 ** np.ceil(np.log2(np.maximum(aw, 1e-30) / 224.0)),
                              1.0).astype(np.float32)
                sw = np.maximum(sw, 2.0 ** -24)
                Wq = (Wp / sw[:, None, None]).astype(ml_dtypes.float8_e4m3fn)
                arrs[f'wp{li}'] = np.ascontiguousarray(
                    Wq.reshape(nwid, kt, 128, Co).transpose(1, 2, 0, 3).reshape(kt, 128, nwid * Co))
                wid_scale[li] = sw
            else:
                arrs[f'wp{li}'] = np.ascontiguousarray(
                    Wp.reshape(nwid, kt, 128, Co).transpose(1, 2, 0, 3).reshape(kt, 128, nwid * Co)
                ).astype(ml_dtypes.bfloat16)
                wid_scale[li] = np.ones(nwid, np.float32)
    NG = st['NG4']
    sv4 = np.ones((128, 2 * NG), np.float32)
    for (w, clo, chi, plo) in st['blocks4']:
        g = plo // 128
        rel = plo - g * 128
        for li in (8, 9):
            sv4[rel:rel + (chi - clo), (li - 8) * NG + g] = wid_scale[li][w]
    arrs['sv4'] = sv4
    for s in (2, 3):
        sv = np.ones((128, 2), np.float32)
        for j, (dc, sc, w) in enumerate(st['slots'][s]):
            for li in (2 * s, 2 * s + 1):
                sv[j, li - 2 * s] = wid_scale[li][w]
        arrs[f'sv{s}'] = sv
    for s in (2, 3):
        slots = st['slots'][s]
        arrs[f'gidx{s}'] = _wrap_idx([sl[1] for sl in slots], _pad16(len(slots)))
    arrs['midx_s'] = _wrap_idx(st['mrg_s'], DELTA)
    arrs['midx_e'] = _wrap_idx(st['mrg_e'], DELTA)
    NS4c = st['NS4c']
    arrs['ident'] = np.eye(128, dtype=np.float32)
    nv = st['SRCWIN'] // 128
    sel = np.zeros((nv, 128, NS4c), np.float32)
    for j, (dc, sc, w) in enumerate(st['compact_slots4']):
        sel[sc // 128, sc % 128, j] = 1.0
    arrs['sel4'] = sel
    for s in (2, 3, 4):
        if s < 4:
            items = list(enumerate(st['slots'][s]))
        else:
            items = list(st['pos_slots4'].items())
        keys = st[f'ohmap{s}']
        oh = np.zeros((len(keys), 128, 512), ml_dtypes.bfloat16)
        pos = {k: i for i, k in enumerate(keys)}
        for j, (dc, sc, w) in items:
            g, c = j // 128, (dc - DELTA) // 512
            oh[pos[(g, c)], j - g * 128, dc - DELTA - c * 512] = 1.0
        arrs[f'oh{s}'] = oh
    return arrs


# --------------------------------------------------------------------------
# Bass program
# --------------------------------------------------------------------------

def _co_dev(li):
    # li9 is cout-sharded across cores: each core computes one 128-wide chunk
    return 128 if li == 9 else COUT[li]


def build_program(st):
    import concourse.tile as tile
    from concourse import bacc, mybir
    import contextlib
    dt = mybir.dt
    AF = mybir.ActivationFunctionType
    ALU = mybir.AluOpType

    nc = bacc.Bacc("TRN2", target_bir_lowering=False, debug=False, num_devices=8)
    NS4c = st['NS4c']
    nv = st['SRCWIN'] // 128

    D = {}
    D['f0'] = nc.dram_tensor("f0", [1, W_COLS], dt.float32r, kind="ExternalInput")
    D['identb'] = nc.dram_tensor("identb", [128, 128], dt.bfloat16, kind="ExternalInput")
    for li in range(10):
        Co = _co_dev(li)
        D[f'wc{li}'] = nc.dram_tensor(f"wc{li}", [CIN[li], Co], dt.float32r, kind="ExternalInput")
        cout_t = (Co + 127) // 128
        D[f'gb{li}'] = nc.dram_tensor(f"gb{li}", [cout_t, 128, 2], dt.float32, kind="ExternalInput")
        s = li // 2
        if s >= 2:
            nwid = len(st['act_off'][s]) + (1 if s == 4 else 0)
            kt = CIN[li] // 128
            pdt = dt.float8e4 if FP8S[s] else dt.bfloat16
            D[f'wp{li}'] = nc.dram_tensor(f"wp{li}", [kt, 128, nwid * Co], pdt, kind="ExternalInput")
    D['sv4'] = nc.dram_tensor("sv4", [128, 2 * st['NG4']], dt.float32, kind="ExternalInput")
    for s in (2, 3):
        D[f'sv{s}'] = nc.dram_tensor(f"sv{s}", [128, 2], dt.float32, kind="ExternalInput")
        pad = _pad16(len(st['slots'][s]))
        D[f'gidx{s}'] = nc.dram_tensor(f"gidx{s}", [128, pad // 16], dt.int16, kind="ExternalInput")
    D['ident'] = nc.dram_tensor("ident", [128, 128], dt.float32r, kind="ExternalInput")
    D['sel4'] = nc.dram_tensor("sel4", [st['SRCWIN'] // 128, 128, NS4c], dt.float32r, kind="ExternalInput")
    D['midx_s'] = nc.dram_tensor("midx_s", [128, DELTA // 16], dt.int16, kind="ExternalInput")
    D['midx_e'] = nc.dram_tensor("midx_e", [128, DELTA // 16], dt.int16, kind="ExternalInput")
    for s in (2, 3, 4):
        nk = len(st[f'ohmap{s}'])
        D[f'oh{s}'] = nc.dram_tensor(f"oh{s}", [nk, 128, 512], dt.bfloat16, kind="ExternalInput")
    D['out'] = nc.dram_tensor("out", [1, 128, N_VOX], dt.float32r, kind="ExternalOutput")

    # stage-4 wave structure and the compact-column split between waves
    blocks4 = st['blocks4']
    NG = st['NG4']
    wave_groups = [list(range(0, min(5, NG))), list(range(5, NG))]
    w0set = set(wave_groups[0])
    cpos_w0 = max((chi for (w, clo, chi, plo) in blocks4 if plo // 128 in w0set), default=0)
    # gather A covers [0, aEnd); gather B covers [bBase, NS4c).  The two may
    # overlap by one 16-granule since cpos_w0 need not be 16-aligned.
    aEnd = ((cpos_w0 + 15) // 16) * 16
    bBase = (cpos_w0 // 16) * 16
    assert aEnd <= NS4c

    with tile.TileContext(nc) as tc:
        ctx = contextlib.ExitStack()
        with ctx:
            fpool = ctx.enter_context(tc.tile_pool(name="f", bufs=1))
            cpool = ctx.enter_context(tc.tile_pool(name="c", bufs=1))
            wpool = ctx.enter_context(tc.tile_pool(name="w", bufs=1))
            spool = ctx.enter_context(tc.tile_pool(name="s", bufs=1))
            ppool = ctx.enter_context(tc.tile_pool(name="p", bufs=1, space="PSUM"))

            # ---- constants (DMAs emitted lazily, spread across early layers
            # so they never queue ahead of a layer's own weight DMAs) ----
            oh_sb = {}
            gidx_sb = {}
            sv_sb = {}

            def load_oh(s, idxs):
                omap = st[f'ohmap{s}']
                for i in idxs:
                    g, c = omap[i]
                    lo, hi = st[f'ohspan{s}'][(g, c)]
                    t = cpool.tile([128, 512], dt.bfloat16, name=f"oh{s}_{i}", bufs=1)
                    nc.sync.dma_start(out=t[:, lo:hi], in_=D[f'oh{s}'][i][:, lo:hi])
                    oh_sb[(s, g, c)] = t

            sel_sb = []

            def load_gidx(s):
                pad = _pad16(len(st['slots'][s]))
                t = cpool.tile([128, pad // 16], dt.int16, name=f"gidx{s}", bufs=1)
                nc.sync.dma_start(out=t[:], in_=D[f'gidx{s}'][:])
                gidx_sb[s] = t

            def load_sel(v0, v1):
                for v in range(v0, min(v1, nv)):
                    t = cpool.tile([128, NS4c], dt.float32r, name=f"sel{v}", bufs=1)
                    nc.sync.dma_start(out=t[:], in_=D['sel4'][v])
                    sel_sb.append(t)

            def load_sv(s):
                wid = 2 * st['NG4'] if s == 4 else 2
                t = cpool.tile([128, wid], dt.float32, name=f"sv{s}", bufs=1)
                nc.sync.dma_start(out=t[:], in_=D[f'sv{s}'][:])
                sv_sb[s] = t
            midx_s = cpool.tile([128, DELTA // 16], dt.int16, name="midx_s", bufs=1)
            midx_e = cpool.tile([128, DELTA // 16], dt.int16, name="midx_e", bufs=1)
            eps_t = cpool.tile([128, 1], dt.float32, name="eps_t", bufs=1)
            nc.vector.memset(eps_t[:], EPS)
            zero32 = cpool.tile([128, DELTA], dt.float32, name="zero32", bufs=1)
            nc.vector.memset(zero32[:], 0.0)
            identb = cpool.tile([128, 128], dt.bfloat16, name="identb", bufs=1)
            nc.sync.dma_start(out=identb[:], in_=D['identb'][:])
            identity = cpool.tile([128, 128], dt.float32r, name="identity", bufs=1)
            nc.sync.dma_start(out=identity[:], in_=D['ident'][:])
            zsq_f = cpool.tile([128, 128], dt.float32, name="zsq_f", bufs=1)
            nc.vector.memset(zsq_f[:], 0.0)
            zsq_r = cpool.tile([128, 128], dt.float32r, name="zsq_r", bufs=1)
            nc.scalar.activation(out=zsq_r[:], in_=zsq_f[:], func=AF.Copy)
            # zero sources for psum-bank "start" dummies: one start=True matmul
            # (or transpose) marks the whole 2KB zero-region; real matmuls then
            # run with start=False so disjoint column ranges don't wipe each
            # other's accumulation.
            zrow_b = cpool.tile([1, 640], dt.bfloat16, name="zrow_b", bufs=1)
            nc.vector.memset(zrow_b[:], 0.0)
            zsq_b = cpool.tile([128, 128], dt.bfloat16, name="zsq_b", bufs=1)
            nc.vector.memset(zsq_b[:], 0.0)
            gidx_dum = cpool.tile([128, 1], dt.int16, name="gidx_dum", bufs=1)
            nc.vector.memset(gidx_dum[:], 0)

            f0t = fpool.tile([128, W_COLS], dt.float32r, name="f0t", tag="f", bufs=8)
            nc.sync.dma_start(out=f0t[:1, :], in_=D['f0'][:])
            fin = [f0t]

            for li in range(10):
                s = li // 2
                Ci, Co = CIN[li], _co_dev(li)
                cin_t = (Ci + 127) // 128
                cout_t = (Co + 127) // 128
                n_bn = st['n_act'][s]
                slots = st['slots'][s] if s >= 2 else []
                nslots = len(slots)
                has_pairs = nslots > 0
                NSp = _pad16(nslots)
                ngroups = (nslots + 127) // 128 if has_pairs else 0

                # center weights + bn params first so their DMAs lead the queue
                wc_sb = []
                for k in range(cin_t):
                    kk = min(128, Ci - k * 128)
                    t = wpool.tile([128, Co], dt.float32r, name=f"wc{li}_{k}", tag="wc", bufs=6)
                    nc.sync.dma_start(out=t[:kk, :], in_=D[f'wc{li}'][k * 128:k * 128 + kk, :])
                    wc_sb.append((t, kk))
                gbt = spool.tile([128, cout_t * 2], dt.float32, name=f"gbt{li}", tag="gbt", bufs=2)
                nc.sync.dma_start(out=gbt[:].rearrange("p (m two) -> p m two", m=cout_t),
                                  in_=D[f'gb{li}'].rearrange("m p two -> p m two"))

                nk4 = len(st['ohmap4'])
                if li == 1:
                    load_gidx(2); load_sv(2); load_oh(2, range(len(st['ohmap2'])))
                    load_gidx(3); load_sv(3); load_oh(3, range(len(st['ohmap3'])))
                elif li == 2:
                    load_sv(4)
                    nc.sync.dma_start(out=midx_s[:], in_=D['midx_s'][:])
                    nc.sync.dma_start(out=midx_e[:], in_=D['midx_e'][:])
                elif li == 3:
                    load_sel(0, 3)
                elif li == 4:
                    load_sel(3, nv); load_oh(4, range(0, nk4 // 3))
                elif li == 5:
                    load_oh(4, range(nk4 // 3, 2 * nk4 // 3))
                elif li == 6:
                    load_oh(4, range(2 * nk4 // 3, nk4))

                Gbf, yt_sb = [], []
                P8 = s >= 2 and FP8S[s]
                pdt = dt.float8e4 if P8 else dt.bfloat16
                T = (cin_t + 1) // 2 if P8 else cin_t
                if has_pairs:
                    if li == 8:
                        # merge deltas into fin[k][:, 0:DELTA]
                        for k in range(cin_t):
                            sg = spool.tile([128, DELTA], dt.float32, name=f"sg{li}_{k}", tag="mrg", bufs=4)
                            eg = spool.tile([128, DELTA], dt.float32, name=f"eg{li}_{k}", tag="mrg", bufs=4)
                            fk32 = fin[k][:].bitcast(dt.float32)
                            nc.gpsimd.ap_gather(out_ap=sg[:], in_ap=fk32, idxs_ap=midx_s[:],
                                                channels=128, num_elems=W_COLS, d=1, num_idxs=DELTA)
                            nc.gpsimd.ap_gather(out_ap=eg[:], in_ap=fk32, idxs_ap=midx_e[:],
                                                channels=128, num_elems=W_COLS, d=1, num_idxs=DELTA)
                            nc.vector.tensor_tensor(out=eg[:], in0=eg[:], in1=sg[:], op=ALU.subtract)
                            nc.vector.tensor_scalar_max(out=eg[:], in0=eg[:], scalar1=0.0)
                            nc.scalar.activation(out=fin[k][:, 0:DELTA], in_=eg[:], func=AF.Copy)
                    # G tiles are [128, ktc, N] (ktc k-tiles packed on the free
                    # axis) so fp8 DoubleRow can contract two k-tiles per pass.
                    def ktc_of(t):
                        return min(2, cin_t - 2 * t) if P8 else 1

                    def k_of(t, i):
                        return 2 * t + i if P8 else t

                    if s < 4:
                        for t_ in range(T):
                            ktc = ktc_of(t_)
                            gd = spool.tile([128, ktc, NSp], pdt, name=f"gd{li}_{t_}", tag="gbf", bufs=5)
                            for i in range(ktc):
                                g32 = spool.tile([128, NSp], dt.float32, name=f"g32_{li}_{t_}_{i}", tag="g32", bufs=2)
                                nc.gpsimd.ap_gather(out_ap=g32[:], in_ap=fin[k_of(t_, i)][:].bitcast(dt.float32),
                                                    idxs_ap=gidx_sb[s][:], channels=128,
                                                    num_elems=W_COLS, d=1, num_idxs=NSp)
                                nc.vector.tensor_copy(out=gd[:, i, :], in_=g32[:])
                            Gbf.append(gd)
                    else:
                        # G via PE transpose + selection matmuls (NOT gpsimd
                        # gathers: blocked gpsimd semaphore waits wake up with
                        # ~11-14us latency, which serializes the whole layer)
                        ft_sb = []
                        for v in range(nv):
                            ftp = ppool.tile([128, 512], dt.float32r, name=f"ftp{li}_{v}", tag="bank", bufs=8)
                            nc.tensor.matmul(ftp[:, 0:16], lhsT=zsq_r[:], rhs=identity[:, :16],
                                             is_transpose=True, start=True, stop=False)
                            for k in range(cin_t):
                                nc.tensor.matmul(ftp[:, k * 128:(k + 1) * 128],
                                                 lhsT=fin[k][:, v * 128:(v + 1) * 128],
                                                 rhs=identity[:], is_transpose=True,
                                                 start=False, stop=(k == cin_t - 1))
                            t = spool.tile([128, cin_t * 128], dt.float32r, name=f"ft{li}_{v}", tag="ft", bufs=9)
                            nc.vector.tensor_copy(out=t[:], in_=ftp[:, :cin_t * 128])
                            ft_sb.append(t)
                        nsc = (NS4c + 511) // 512
                        for k in range(cin_t):
                            gb_ = spool.tile([128, NS4c], dt.bfloat16, name=f"gbf{li}_{k}", tag="gbf", bufs=5)
                            Gbf.append(gb_)
                            for c in range(nsc):
                                w_ = min(512, NS4c - c * 512)
                                gp = ppool.tile([128, 512], dt.float32, name=f"gp{li}_{k}_{c}", tag="bank", bufs=8)
                                for v in range(nv):
                                    nc.tensor.matmul(gp[:, :w_], lhsT=ft_sb[v][:, k * 128:(k + 1) * 128],
                                                     rhs=sel_sb[v][:, c * 512:c * 512 + w_],
                                                     start=(v == 0), stop=(v == nv - 1))
                                nc.vector.tensor_copy(out=gb_[:, c * 512:c * 512 + w_], in_=gp[:, :w_])

                    if s == 4:
                        # transposed pair GEMM: YT[slot, cout] directly, per
                        # wid-block (32-padded starts).  Groups processed in
                        # waves of <=5 psum banks; weights DMA'd per wave.
                        yt_sb = [None] * NG
                        for wg in wave_groups:
                            if not wg:
                                continue
                            blks = [b for b in blocks4 if b[3] // 128 in wg]
                            wlo = min(b[0] for b in blks)
                            whi = max(b[0] for b in blks) + 1
                            ytp = {}
                            for g in wg:
                                t = ppool.tile([128, 512], dt.float32, name=f"ytp{li}_{g}", tag="bank", bufs=8)
                                nc.tensor.matmul(t[:, :16], lhsT=zrow_b[:1, :128],
                                                 rhs=zrow_b[:1, 128:144], start=True, stop=False)
                                ytp[g] = t
                            wmid = (wlo + whi + 1) // 2
                            for k in range(cin_t):
                                for h, (h0, h1) in enumerate(((wlo, wmid), (wmid, whi))):
                                    if h1 <= h0:
                                        continue
                                    wpt = wpool.tile([128, h1 - h0, Co], dt.bfloat16,
                                                     name=f"wpt{li}_{k}_{wg[0]}_{h}", tag=f"wpt{h}", bufs=2)
                                    nc.sync.dma_start(out=wpt[:],
                                                      in_=D[f'wp{li}'][k, :, h0 * Co:h1 * Co].rearrange(
                                                          "p (w c) -> p w c", w=h1 - h0))
                                    for (w, clo, chi, plo) in blks:
                                        if not (h0 <= w < h1):
                                            continue
                                        g = plo // 128
                                        rel = plo - g * 128
                                        nc.tensor.matmul(
                                            ytp[g][rel:rel + (chi - clo), :Co],
                                            lhsT=Gbf[k][:, clo:chi],
                                            rhs=wpt[:, w - h0],
                                            start=False, stop=(k == cin_t - 1))
                            for g in wg:
                                t = spool.tile([128, Co], dt.bfloat16, name=f"yt{li}_{g}", tag="yt", bufs=10)
                                nc.scalar.activation(out=t[:], in_=ytp[g][:, :Co], func=AF.Copy,
                                                     scale=sv_sb[4][:, (li - 8) * NG + g:(li - 8) * NG + g + 1])
                                yt_sb[g] = t
                        ngroups = NG
                    else:
                        # small stages: non-transposed Y then PE transpose
                        ranges = slot_ranges(slots)
                        nsc = (NSp + 511) // 512
                        yp = [[None] * nsc for _ in range(cout_t)]
                        for m in range(cout_t):
                            for c in range(nsc):
                                yp[m][c] = ppool.tile([128, 512], dt.float32, name=f"yp{li}_{m}_{c}", tag="bank", bufs=8)
                                nc.tensor.matmul(yp[m][c][:, :16], lhsT=zrow_b[:1, :128],
                                                 rhs=zrow_b[:1, 128:144], start=True, stop=False)
                        nwid = len(st['act_off'][s])
                        nw0 = (nwid + 1) // 2
                        for t_ in range(T):
                            ktc = ktc_of(t_)
                            for h, (w0, w1) in enumerate(((0, nw0), (nw0, nwid))):
                                nh = w1 - w0
                                if nh <= 0:
                                    continue
                                wpt = wpool.tile([128, nh, ktc, Co], pdt, name=f"wpt{li}_{t_}_{h}", tag=f"wpt{h}", bufs=2)
                                for i in range(ktc):
                                    nc.sync.dma_start(out=wpt[:, :, i, :],
                                                      in_=D[f'wp{li}'][(2 * t_ if P8 else t_) + i, :,
                                                                       w0 * Co:w1 * Co].rearrange(
                                                          "p (w c) -> p w c", w=nh))
                                for (w, lo, hi) in ranges:
                                    if not (w0 <= w < w1):
                                        continue
                                    c = lo // 512
                                    for m in range(cout_t):
                                        pm = min(128, Co - m * 128)
                                        last = t_ == T - 1
                                        if P8 and ktc == 2:
                                            nc.tensor.matmul(
                                                yp[m][c][:pm, lo - c * 512:hi - c * 512],
                                                lhsT=wpt[:, w - w0, :, m * 128:m * 128 + pm],
                                                rhs=Gbf[t_][:, :, lo:hi],
                                                perf_mode=mybir.MatmulPerfMode.DoubleRow,
                                                start=False, stop=last)
                                        else:
                                            for i in range(ktc):
                                                nc.tensor.matmul(
                                                    yp[m][c][:pm, lo - c * 512:hi - c * 512],
                                                    lhsT=wpt[:, w - w0, i, m * 128:m * 128 + pm],
                                                    rhs=Gbf[t_][:, i, lo:hi],
                                                    start=False, stop=last and i == ktc - 1)
                        NSG = ngroups * 128
                        ysb = []
                        for m in range(cout_t):
                            t = spool.tile([128, NSG], dt.bfloat16, name=f"ysb{li}_{m}", tag="ysb", bufs=5)
                            if NSG > NSp:
                                nc.vector.memset(t[:, NSp:], 0.0)
                            for c in range(nsc):
                                w_ = min(512, NSp - c * 512)
                                nc.vector.tensor_copy(out=t[:, c * 512:c * 512 + w_], in_=yp[m][c][:, :w_])
                            ysb.append(t)
                        for g in range(ngroups):
                            ytp = ppool.tile([128, 512], dt.bfloat16, name=f"ytp{li}_{g}", tag="bank", bufs=8)
                            nc.tensor.matmul(ytp[:, 0:16], lhsT=zsq_b[:], rhs=identb[:, :16],
                                             is_transpose=True, start=True, stop=False)
                            for m in range(cout_t):
                                nc.tensor.matmul(ytp[:, m * 128:(m + 1) * 128],
                                                 lhsT=ysb[m][:, g * 128:(g + 1) * 128],
                                                 rhs=identb[:], is_transpose=True,
                                                 start=False, stop=(m == cout_t - 1))
                            t = spool.tile([128, Co], dt.bfloat16, name=f"yt{li}_{g}", tag="yt", bufs=10)
                            nc.scalar.activation(out=t[:], in_=ytp[:, :Co], func=AF.Copy,
                                                 scale=sv_sb[s][:, li - 2 * s:li - 2 * s + 1])
                            yt_sb.append(t)

                # ---- center GEMM + scatter + BN ----
                fout = []
                for m in range(cout_t):
                    pm = min(128, Co - m * 128)
                    fo = fpool.tile([128, W_COLS], dt.float32r, name=f"f{li}_{m}", tag="f", bufs=8)
                    fout.append(fo)
                    ps_c = []
                    for c in range(4):
                        ps = ppool.tile([128, 512], dt.float32, name=f"ps{li}_{m}_{c}", tag="bank", bufs=8)
                        ps_c.append(ps)
                        mms = [('c', k) for k in range(cin_t)]
                        if has_pairs:
                            mms += [('s', g) for g in range(ngroups) if (s, g, c) in oh_sb]
                        for i, (kind, a) in enumerate(mms):
                            last = i == len(mms) - 1
                            if kind == 'c':
                                wt, kk = wc_sb[a]
                                nc.tensor.matmul(
                                    ps[:pm, :], lhsT=wt[:kk, m * 128:m * 128 + pm],
                                    rhs=fin[a][:kk, DELTA + c * 512:DELTA + (c + 1) * 512],
                                    start=(a == 0), stop=last)
                            else:
                                lo, hi = st[f'ohspan{s}'][(a, c)]
                                nc.tensor.matmul(
                                    ps[:pm, lo:hi], lhsT=yt_sb[a][:, m * 128:m * 128 + pm],
                                    rhs=oh_sb[(s, a, c)][:, lo:hi], start=False, stop=last)
                    stats = spool.tile([128, 4, 6], dt.float32, name=f"bs{li}_{m}", tag="bs", bufs=3)
                    for c in range(4):
                        hi = min(512, n_bn - c * 512)
                        nc.vector.bn_stats(out=stats[:pm, c, :], in_=ps_c[c][:pm, :hi])
                    mv = spool.tile([128, 2], dt.float32, name=f"mv{li}_{m}", tag="mv", bufs=3)
                    nc.vector.bn_aggr(out=mv[:pm, :], in_=stats[:pm, :, :])
                    scale = spool.tile([128, 1], dt.float32, name=f"sc{li}_{m}", tag="sc", bufs=3)
                    shift = spool.tile([128, 1], dt.float32, name=f"sh{li}_{m}", tag="sh", bufs=3)
                    tmp = spool.tile([128, 1], dt.float32, name=f"tp{li}_{m}", tag="tp", bufs=3)
                    nc.scalar.activation(out=scale[:pm, :], in_=mv[:pm, 1:2], func=AF.Abs_reciprocal_sqrt,
                                         bias=eps_t[:pm, :], scale=1.0)
                    nc.vector.tensor_tensor(out=scale[:pm, :], in0=scale[:pm, :],
                                            in1=gbt[:pm, 2 * m:2 * m + 1], op=ALU.mult)
                    nc.vector.tensor_tensor(out=tmp[:pm, :], in0=mv[:pm, 0:1],
                                            in1=scale[:pm, :], op=ALU.mult)
                    nc.vector.tensor_tensor(out=shift[:pm, :], in0=gbt[:pm, 2 * m + 1:2 * m + 2],
                                            in1=tmp[:pm, :], op=ALU.subtract)
                    nc.scalar.activation(out=fo[:, 0:DELTA], in_=zero32[:], func=AF.Copy)
                    for c in range(4):
                        nc.scalar.activation(out=fo[:pm, DELTA + c * 512:DELTA + (c + 1) * 512],
                                             in_=ps_c[c][:pm, :], func=AF.Relu,
                                             bias=shift[:pm, :], scale=scale[:pm, :])
                        if li == 9:
                            nc.sync.dma_start(out=D['out'][0][:, c * 512:(c + 1) * 512],
                                              in_=fo[:, DELTA + c * 512:DELTA + (c + 1) * 512])
                    # paced gpsimd keep-awake: a dependency-chained no-op gather
                    # so the gpsimd sequencer's sleep lands on a harmless dummy
                    # instead of the next real gather (wakeup costs ~11-14us)
                    gdum = spool.tile([128, 16], dt.float32, name=f"gd{li}_{m}", tag="gdum", bufs=2)
                    nc.gpsimd.ap_gather(out_ap=gdum[:], in_ap=stats[:, 0, :], idxs_ap=gidx_dum[:],
                                        channels=128, num_elems=6, d=1, num_idxs=16)
                fin = fout
    nc.compile()
    return nc


# --------------------------------------------------------------------------
# entry point
# --------------------------------------------------------------------------

_CACHE = {}


def _get(coors):
    key = coors.tobytes()
    if key not in _CACHE:
        st = build_structure(coors)
        nc = build_program(st)
        _CACHE[key] = (st, nc)
    return _CACHE[key]


def run_device(st, nc, inputs, trace=False):
    from concourse.bass_utils import run_bass_kernel_spmd
    arrs = pack_structure(st, inputs)
    base = {k: np.ascontiguousarray(v) for k, v in arrs.items()}
    # li9 is cout-sharded: core c computes output channels of chunk c//2
    in_maps = []
    nwid9 = base['wp9'].shape[2] // COUT[9]
    wp9v = base['wp9'].reshape(base['wp9'].shape[0], 128, nwid9, COUT[9])
    for c in range(8):
        m = c // 2
        d = dict(base)
        d['wc9'] = np.ascontiguousarray(base['wc9'][:, m * 128:(m + 1) * 128])
        d['wp9'] = np.ascontiguousarray(
            wp9v[:, :, :, m * 128:(m + 1) * 128]).reshape(wp9v.shape[0], 128, nwid9 * 128)
        d['gb9'] = np.ascontiguousarray(base['gb9'][m:m + 1])
        in_maps.append(d)
    res = run_bass_kernel_spmd(nc, in_maps, list(range(8)), trace=trace)
    return res


def gather_f9(res):
    # rows m*128:(m+1)*128 of f9 live on cores 2m and 2m+1
    return np.concatenate([res.results[2 * m]['out'].reshape(128, N_VOX)
                           for m in range(4)], axis=0)


def postprocess(st, f9):
    prep = st['prep']
    col = st['col'] - DELTA
    out_vals = {}
    for members in prep['pool_groups'][4]:
        surv = members[0]
        val = f9[:, col[surv]].copy()
        for e in members[1:]:
            val = np.maximum(val, f9[:, col[e]])
        out_vals[surv] = val
    D5, H5, W5 = DIMS[5]
    dense = np.zeros((BATCH, D5, H5, W5, 512), np.float32)
    fa = prep['final_active']; fc = prep['final_coords']; fb = prep['final_bidx']
    for v in np.where(fa)[0]:
        val = out_vals.get(int(v))
        if val is None:
            val = f9[:, col[v]]
        dense[fb[v], fc[v][0], fc[v][1], fc[v][2]] += val
    return dense.transpose(0, 4, 1, 2, 3).reshape(BATCH, 512 * D5, H5, W5)


def kernel(**inputs):
    coors = np.asarray(inputs['coors'])
    st, nc = _get(coors)
    res = run_device(st, nc, inputs)
    return postprocess(st, gather_f9(res))



# revision 48
# speedup vs baseline: 1.1503x; 1.1503x over previous
# Trainium2 Bass kernel for nn_PointGridExtractor (submanifold sparse 3D CNN).
#
# The voxel grid is astronomically sparse (2048 active sites in up to 2.4e11
# cells), so each submanifold conv is a center-tap GEMM plus a small set of
# real neighbor pairs (0/0/34/128/838 pairs for stages 0..4 on this input).
# All index structure depends only on `coors` and is computed host-side; the
# device does dense GEMM/BN/ReLU work:
#   - activations in SBUF as [channel_partitions, voxel_column] (float32r)
#   - center conv: PE matmuls, weights stationary
#   - neighbor pairs: gather src columns (GPSIMD ap_gather for small stages;
#     PE transpose of the source window + selection matmul for stage 4),
#     per-offset transposed GEMMs (bf16 weights halve the dominant ~40MB
#     pair-weight stream), one-hot scatter matmul accumulates into the main
#     conv PSUM
#   - pool3's 18 max-merges become virtual "delta" pairs
#     (delta = relu(extra - survivor) feeds the next conv); pool4 merges and
#     the final dense scatter run on the host
#   - BN stats via bn_stats/bn_aggr over a contiguous active-column prefix
#     (column order puts dead columns last); fused relu(scale*x+shift) on ACT
# The 8 cores run the identical program: at this size every cross-core
# exchange (collectives measured at 25-50us each) costs more than local
# recompute, so full replication is the fastest distribution.
import numpy as np
import ml_dtypes

N_VOX = 2048
DELTA = 32              # columns [0, DELTA): merge-delta slots + zero pad
W_COLS = DELTA + N_VOX
ZERO_COL = DELTA - 1    # always-zero column (gather pad target)
DIMS = [(216, 7992, 7056), (72, 2664, 2352), (24, 888, 784), (8, 444, 392), (4, 222, 196), (2, 111, 98)]
KS = [7, 7, 7, 7, 5, 5, 3, 3, 3, 3]
POOLS = [(3, 3, 3), (3, 3, 3), (3, 2, 2), (2, 2, 2), (2, 2, 2)]
CIN = [1, 64, 64, 128, 128, 256, 256, 384, 384, 512]
COUT = [64, 64, 128, 128, 256, 256, 384, 384, 512, 512]
BIG = np.int64(2 ** 62)
EPS = 1e-4
BATCH = 2
# pair-GEMM dtype per stage: fp8e4 weights/activations (with per-wid
# power-of-2 scales folded into the yt copy) vs bf16.  fp8 measured ~2.6%
# acc error per stage-4 layer (pair sums are random-walk sized; quant noise
# does not average out) — too lossy for the 2e-2 budget, so bf16.
FP8S = {2: False, 3: False, 4: False}


def _encode(c, b, dims):
    D, H, W = dims
    return ((b * np.int64(D) + c[:, 0]) * H + c[:, 1]) * W + c[:, 2]


def _offsets(k):
    r = k // 2
    g = np.arange(-r, r + 1)
    return np.stack(np.meshgrid(g, g, g, indexing='ij'), -1).reshape(-1, 3).astype(np.int64)


# --------------------------------------------------------------------------
# host index pipeline
# --------------------------------------------------------------------------

def host_prep(coors):
    coords = coors[:, 1:4].astype(np.int64)
    bidx = coors[:, 0].astype(np.int64)
    N = coords.shape[0]
    assert N == N_VOX
    active = np.ones(N, bool)
    stage_pairs = []   # s -> (dst, src, noncenter_off_index)
    stage_active = []
    pool_groups = []
    for s in range(5):
        stage_active.append(active.copy())
        dims = DIMS[s]
        k = KS[2 * s]
        keys = np.where(active, _encode(coords, bidx, dims), BIG)
        order = np.argsort(keys, kind='stable')
        skeys = keys[order]
        offs = _offsets(k)
        ctr = len(offs) // 2
        dsts, srcs, oids = [], [], []
        act_idx = np.where(active)[0]
        noff = 0
        for oi, off in enumerate(offs):
            if oi == ctr:
                continue
            nc_ = coords[act_idx] + off[None, :]
            ok = ((nc_ >= 0) & (nc_ < np.array(dims)[None, :])).all(1)
            nkey = np.where(ok, _encode(nc_, bidx[act_idx], dims), np.int64(-1))
            pos = np.clip(np.searchsorted(skeys, nkey), 0, N - 1)
            hit = ok & (skeys[pos] == nkey)
            if hit.any():
                dsts.append(act_idx[hit])
                srcs.append(order[pos[hit]])
                oids.append(np.full(int(hit.sum()), noff))
            noff += 1
        if dsts:
            stage_pairs.append((np.concatenate(dsts), np.concatenate(srcs),
                                np.concatenate(oids)))
        else:
            stage_pairs.append((np.zeros(0, np.int64),) * 3)
        stride = np.array(POOLS[s], np.int64)
        odims = DIMS[s + 1]
        pc = coords // stride[None, :]
        pkeys = np.where(active, _encode(pc, bidx, odims), BIG)
        groups = {}
        for v in act_idx:
            groups.setdefault(int(pkeys[v]), []).append(int(v))
        new_active = np.zeros(N, bool)
        gmulti = []
        for key, members in groups.items():
            surv = members[0]
            new_active[surv] = True
            D, H, W = odims
            w = key % W; t = key // W
            h = t % H; t = t // H
            d = t % D; b = t // D
            coords[surv] = (d, h, w)
            bidx[surv] = b
            if len(members) > 1:
                gmulti.append(members)
        pool_groups.append(gmulti)
        active = new_active
    return dict(stage_pairs=stage_pairs, stage_active=stage_active,
                pool_groups=pool_groups, final_active=active,
                final_coords=coords, final_bidx=bidx)


def _pad16(n):
    return max(16, ((n + 15) // 16) * 16)


def build_structure(coors):
    prep = host_prep(coors)
    sp = prep['stage_pairs']
    pg3 = prep['pool_groups'][3]
    n_mrg = len(pg3)
    assert n_mrg <= DELTA - 1, n_mrg
    assert all(len(m) == 2 for m in pg3), "pool3 groups expected size 2"
    dead3 = np.array(sorted(e for m in pg3 for e in m[1:]), np.int64)
    surv3 = {m[0]: g for g, m in enumerate(pg3)}

    # ---- column order: [0,DELTA) deltas | srcs4 | dst4-only | rest | dead3
    src4 = np.unique(sp[4][1]) if len(sp[4][0]) else np.zeros(0, np.int64)
    dst4 = np.unique(np.concatenate([sp[4][0], np.fromiter(surv3.keys(), np.int64, len(surv3))])) if (len(sp[4][0]) or surv3) else np.zeros(0, np.int64)
    dead_set = set(dead3.tolist())
    assert not (set(src4.tolist()) | set(dst4.tolist())) & dead_set
    ordered, seen = [], set(dead_set)
    for v in src4:
        if int(v) not in seen:
            ordered.append(int(v)); seen.add(int(v))
    n_src4 = len(ordered)
    for v in dst4:
        if int(v) not in seen:
            ordered.append(int(v)); seen.add(int(v))
    for v in range(N_VOX):
        if v not in seen:
            ordered.append(v); seen.add(v)
    ordered.extend(int(x) for x in dead3)
    assert len(ordered) == N_VOX
    col = np.zeros(N_VOX, np.int64)
    for pos, v in enumerate(ordered):
        col[v] = DELTA + pos
    SRCWIN = ((DELTA + n_src4 + 127) // 128) * 128

    n_act = [int(a.sum()) for a in prep['stage_active']]
    assert n_act[:4] == [N_VOX] * 4
    act4_cols = np.sort(col[prep['stage_active'][4]])
    assert act4_cols[-1] == DELTA + n_act[4] - 1, "active prefix broken"

    def mk_slots_small(s):
        d, sv, oi = sp[s]
        order = np.argsort(oi, kind='stable')
        d, sv, oi = d[order], sv[order], oi[order]
        wids = np.unique(oi)
        wid_of = {int(o): i for i, o in enumerate(wids)}
        slots = [(int(col[a]), int(col[b]), wid_of[int(o)]) for a, b, o in zip(d, sv, oi)]
        return slots, [int(o) for o in wids]

    slots2, act_off2 = mk_slots_small(2)
    slots3, act_off3 = mk_slots_small(3)

    d4, s4, o4 = sp[4]
    wids4 = np.unique(o4)
    wid_of4 = {int(o): i for i, o in enumerate(wids4)}
    CENTER_WID4 = len(wids4)
    raw = []
    for a, b, o in zip(d4, s4, o4):
        raw.append((int(col[a]), int(col[b]), wid_of4[int(o)]))
        if int(b) in surv3:
            raw.append((int(col[a]), surv3[int(b)], wid_of4[int(o)]))
    for sv_vox, g in surv3.items():
        raw.append((int(col[sv_vox]), g, CENTER_WID4))
    slots4 = sorted(raw, key=lambda t: t[2])
    act_off4 = [int(o) for o in wids4]

    # ---- stage-4 packed block layout (transposed pair GEMM) ----
    # Each wid gets a 32-padded block; blocks may not cross 128-group
    # boundaries and must start at 0/32/64 within a group (PE tile_position
    # constraint; 96 is not a legal base).  Sizes <=64 assumed (assert).
    by_wid = {}
    for (dc, sc, w) in slots4:
        by_wid.setdefault(w, []).append((dc, sc))
    # sizes padded to 32; legal block starts within a group: 0/32/64, with
    # 64-blocks only at 0/64 and >=96-blocks only at 0.  Optimal grouping:
    # (32,32,64) full groups, then (64,64), then 32-triples (32 wasted).
    psz = {w: ((len(by_wid[w]) + 31) // 32) * 32 for w in by_wid}
    assert all(v <= 128 for v in psz.values())
    b96 = [w for w in by_wid if psz[w] >= 96]
    b64 = [w for w in by_wid if psz[w] == 64]
    b32 = [w for w in by_wid if psz[w] == 32]
    group_layouts = []   # list of [(wid, start, padded)]
    for w in b96:
        group_layouts.append([(w, 0, psz[w])])
    while b64 and len(b32) >= 2:
        a, b = b32.pop(), b32.pop()
        c = b64.pop()
        group_layouts.append([(a, 0, 32), (b, 32, 32), (c, 64, 64)])
    while len(b64) >= 2:
        c, d = b64.pop(), b64.pop()
        group_layouts.append([(c, 0, 64), (d, 64, 64)])
    if b64:
        g = [(b64.pop(), 0, 64)]
        if b32:
            g.append((b32.pop(), 64, 32))
        group_layouts.append(g)
    while b32:
        g = []
        for start in (0, 32, 64):
            if b32:
                g.append((b32.pop(), start, 32))
        group_layouts.append(g)
    # blocks: (wid, compact_lo, compact_hi, padded_lo).  G is built compact
    # (sel has NS4c columns); the pair GEMM writes the block at its padded
    # psum position.  Pad rows hold stale-but-finite psum garbage which the
    # zero one-hot rows null out in the scatter.
    blocks = []
    pos_slots4 = {}   # padded position -> (dst, src, wid), for one-hots
    wid_neworder = []
    cpos = 0
    for gi, gl in enumerate(group_layouts):
        for (w, start, padded) in gl:
            n = len(by_wid[w])
            blocks.append((w, cpos, cpos + n, gi * 128 + start))
            wid_neworder.append(w)
            for i, (dc, sc) in enumerate(by_wid[w]):
                pos_slots4[gi * 128 + start + i] = (dc, sc, w)
            cpos += n
    NG4 = len(group_layouts)
    NS4c = _pad16(cpos)
    # compact slot list for sel
    compact_slots4 = []
    for (w, clo, chi, plo) in blocks:
        compact_slots4.extend((dc, sc, w) for (dc, sc) in by_wid[w])
    widmap = {w: i for i, w in enumerate(wid_neworder)}
    blocks = [(widmap[w], clo, chi, plo) for (w, clo, chi, plo) in blocks]
    pairlist4 = [(-1 if w == CENTER_WID4 else act_off4[w]) for w in wid_neworder]

    mrg_s = np.full(DELTA, ZERO_COL, np.int64)
    mrg_e = np.full(DELTA, ZERO_COL, np.int64)
    for svx, g in surv3.items():
        mrg_s[g] = col[svx]
    for m in pg3:
        mrg_e[surv3[m[0]]] = col[m[1]]

    st = dict(prep=prep, col=col, SRCWIN=SRCWIN, n_act=n_act,
              slots=[None, None, slots2, slots3, slots4],
              act_off=[None, None, act_off2, act_off3, act_off4],
              center_wid4=CENTER_WID4, mrg_s=mrg_s, mrg_e=mrg_e, n_mrg=n_mrg,
              blocks4=blocks, pos_slots4=pos_slots4, pairlist4=pairlist4,
              compact_slots4=compact_slots4, NS4c=NS4c,
              NG4=NG4, NS4=NG4 * 128)

    # one-hot scatter map + dst spans: per stage, (group, chunk) -> [jmin, jend)
    def mk_ohmap(items, tag):
        hits = {}
        for j, (dc, sc, w) in items:
            key = (j // 128, (dc - DELTA) // 512)
            rel = dc - DELTA - key[1] * 512
            lo, hi = hits.get(key, (512, 0))
            hits[key] = (min(lo, rel), max(hi, rel + 1))
        st[f'ohmap{tag}'] = sorted(hits.keys())
        st[f'ohspan{tag}'] = hits
    mk_ohmap(list(enumerate(slots2)), 2)
    mk_ohmap(list(enumerate(slots3)), 3)
    mk_ohmap(list(pos_slots4.items()), 4)
    return st


def slot_ranges(slots):
    """Contiguous (wid, lo, hi) ranges split at 128-group boundaries."""
    n = len(slots)
    ranges = []
    i = 0
    while i < n:
        j = i
        w = slots[i][2]
        while j < n and slots[j][2] == w:
            j += 1
        lo = i
        while lo < j:
            hi = min(j, ((lo // 128) + 1) * 128)
            ranges.append((w, lo, hi))
            lo = hi
        i = j
    return ranges


# --------------------------------------------------------------------------
# device input packing
# --------------------------------------------------------------------------

def _wrap_idx(idxs, pad_to):
    out = np.zeros((128, pad_to // 16), np.int16)
    for j in range(pad_to):
        v = idxs[j] if j < len(idxs) else ZERO_COL
        out[np.arange(8) * 16 + (j % 16), j // 16] = v
    return out


def pack_structure(st, inputs):
    col = st['col']
    arrs = {}
    wid_scale = {}
    vf = np.asarray(inputs['voxel_features'], np.float32)
    f0 = np.zeros((1, W_COLS), np.float32)
    f0[0, col] = vf[:, 0]
    arrs['f0'] = f0
    arrs['identb'] = np.eye(128, dtype=ml_dtypes.bfloat16)
    for li in range(10):
        W = np.asarray(inputs[f'conv_w_{li}'], np.float32)
        k3 = KS[li] ** 3
        ctr = k3 // 2
        arrs[f'wc{li}'] = np.ascontiguousarray(W[ctr])
        g = np.asarray(inputs[f'bn_g_{li}'], np.float32)
        b = np.asarray(inputs[f'bn_b_{li}'], np.float32)
        cout_t = (COUT[li] + 127) // 128
        gb = np.zeros((cout_t, 128, 2), np.float32)
        gb[:, :, 0].reshape(-1)[:COUT[li]] = g
        gb[:, :, 1].reshape(-1)[:COUT[li]] = b
        arrs[f'gb{li}'] = gb
        s = li // 2
        if s >= 2:
            offs_nc = [oi for oi in range(k3) if oi != ctr]
            if s < 4:
                act = st['act_off'][s]
                mats = [W[offs_nc[oi]] for oi in act]
            else:
                mats = [(W[ctr] if oi < 0 else W[offs_nc[oi]]) for oi in st['pairlist4']]
            Wp = np.stack(mats)
            nwid, Ci, Co = Wp.shape
            kt = Ci // 128
            # HBM layout [kt, 128, nwid*Co]: per-partition lines are contiguous
            # nwid*Co runs so the wpt DMA moves large descriptors
            if FP8S[s]:
                aw = np.abs(Wp).reshape(nwid, -1).max(1)
                sw = np.where(aw > 0, 2.0 ** np.ceil(np.log2(np.maximum(aw, 1e-30) / 224.0)),
                              1.0).astype(np.float32)
                sw = np.maximum(sw, 2.0 ** -24)
                Wq = (Wp / sw[:, None, None]).astype(ml_dtypes.float8_e4m3fn)
                arrs[f'wp{li}'] = np.ascontiguousarray(
                    Wq.reshape(nwid, kt, 128, Co).transpose(1, 2, 0, 3).reshape(kt, 128, nwid * Co))
                wid_scale[li] = sw
            else:
                arrs[f'wp{li}'] = np.ascontiguousarray(
                    Wp.reshape(nwid, kt, 128, Co).transpose(1, 2, 0, 3).reshape(kt, 128, nwid * Co)
                ).astype(ml_dtypes.bfloat16)
                wid_scale[li] = np.ones(nwid, np.float32)
    NG = st['NG4']
    sv4 = np.ones((128, 2 * NG), np.float32)
    for (w, clo, chi, plo) in st['blocks4']:
        g = plo // 128
        rel = plo - g * 128
        for li in (8, 9):
            sv4[rel:rel + (chi - clo), (li - 8) * NG + g] = wid_scale[li][w]
    arrs['sv4'] = sv4
    for s in (2, 3):
        sv = np.ones((128, 2), np.float32)
        for j, (dc, sc, w) in enumerate(st['slots'][s]):
            for li in (2 * s, 2 * s + 1):
                sv[j, li - 2 * s] = wid_scale[li][w]
        arrs[f'sv{s}'] = sv
    for s in (2, 3):
        slots = st['slots'][s]
        arrs[f'gidx{s}'] = _wrap_idx([sl[1] for sl in slots], _pad16(len(slots)))
    arrs['midx_s'] = _wrap_idx(st['mrg_s'], DELTA)
    arrs['midx_e'] = _wrap_idx(st['mrg_e'], DELTA)
    NS4c = st['NS4c']
    arrs['ident'] = np.eye(128, dtype=np.float32)
    nv = st['SRCWIN'] // 128
    sel = np.zeros((nv, 128, NS4c), np.float32)
    for j, (dc, sc, w) in enumerate(st['compact_slots4']):
        sel[sc // 128, sc % 128, j] = 1.0
    arrs['sel4'] = sel
    for s in (2, 3, 4):
        if s < 4:
            items = list(enumerate(st['slots'][s]))
        else:
            items = list(st['pos_slots4'].items())
        keys = st[f'ohmap{s}']
        oh = np.zeros((len(keys), 128, 512), ml_dtypes.bfloat16)
        pos = {k: i for i, k in enumerate(keys)}
        for j, (dc, sc, w) in items:
            g, c = j // 128, (dc - DELTA) // 512
            oh[pos[(g, c)], j - g * 128, dc - DELTA - c * 512] = 1.0
        arrs[f'oh{s}'] = oh
    return arrs


# --------------------------------------------------------------------------
# Bass program
# --------------------------------------------------------------------------

def _co_dev(li):
    # li9 is cout-sharded across cores: each core computes one 128-wide chunk
    return 128 if li == 9 else COUT[li]


def build_program(st):
    import concourse.tile as tile
    from concourse import bacc, mybir
    import contextlib
    dt = mybir.dt
    AF = mybir.ActivationFunctionType
    ALU = mybir.AluOpType

    nc = bacc.Bacc("TRN2", target_bir_lowering=False, debug=False, num_devices=8)
    NS4c = st['NS4c']
    nv = st['SRCWIN'] // 128

    D = {}
    D['f0'] = nc.dram_tensor("f0", [1, W_COLS], dt.float32r, kind="ExternalInput")
    D['identb'] = nc.dram_tensor("identb", [128, 128], dt.bfloat16, kind="ExternalInput")
    for li in range(10):
        Co = _co_dev(li)
        D[f'wc{li}'] = nc.dram_tensor(f"wc{li}", [CIN[li], Co], dt.float32r, kind="ExternalInput")
        cout_t = (Co + 127) // 128
        D[f'gb{li}'] = nc.dram_tensor(f"gb{li}", [cout_t, 128, 2], dt.float32, kind="ExternalInput")
        s = li // 2
        if s >= 2:
            nwid = len(st['act_off'][s]) + (1 if s == 4 else 0)
            kt = CIN[li] // 128
            pdt = dt.float8e4 if FP8S[s] else dt.bfloat16
            D[f'wp{li}'] = nc.dram_tensor(f"wp{li}", [kt, 128, nwid * Co], pdt, kind="ExternalInput")
    D['sv4'] = nc.dram_tensor("sv4", [128, 2 * st['NG4']], dt.float32, kind="ExternalInput")
    for s in (2, 3):
        D[f'sv{s}'] = nc.dram_tensor(f"sv{s}", [128, 2], dt.float32, kind="ExternalInput")
        pad = _pad16(len(st['slots'][s]))
        D[f'gidx{s}'] = nc.dram_tensor(f"gidx{s}", [128, pad // 16], dt.int16, kind="ExternalInput")
    D['ident'] = nc.dram_tensor("ident", [128, 128], dt.float32r, kind="ExternalInput")
    D['sel4'] = nc.dram_tensor("sel4", [st['SRCWIN'] // 128, 128, NS4c], dt.float32r, kind="ExternalInput")
    D['midx_s'] = nc.dram_tensor("midx_s", [128, DELTA // 16], dt.int16, kind="ExternalInput")
    D['midx_e'] = nc.dram_tensor("midx_e", [128, DELTA // 16], dt.int16, kind="ExternalInput")
    for s in (2, 3, 4):
        nk = len(st[f'ohmap{s}'])
        D[f'oh{s}'] = nc.dram_tensor(f"oh{s}", [nk, 128, 512], dt.bfloat16, kind="ExternalInput")
    D['out'] = nc.dram_tensor("out", [1, 128, N_VOX], dt.float32r, kind="ExternalOutput")

    # stage-4 wave structure and the compact-column split between waves
    blocks4 = st['blocks4']
    NG = st['NG4']
    wave_groups = [list(range(0, min(5, NG))), list(range(5, NG))]
    w0set = set(wave_groups[0])
    cpos_w0 = max((chi for (w, clo, chi, plo) in blocks4 if plo // 128 in w0set), default=0)
    # gather A covers [0, aEnd); gather B covers [bBase, NS4c).  The two may
    # overlap by one 16-granule since cpos_w0 need not be 16-aligned.
    aEnd = ((cpos_w0 + 15) // 16) * 16
    bBase = (cpos_w0 // 16) * 16
    assert aEnd <= NS4c

    with tile.TileContext(nc) as tc:
        ctx = contextlib.ExitStack()
        with ctx:
            fpool = ctx.enter_context(tc.tile_pool(name="f", bufs=1))
            cpool = ctx.enter_context(tc.tile_pool(name="c", bufs=1))
            wpool = ctx.enter_context(tc.tile_pool(name="w", bufs=1))
            spool = ctx.enter_context(tc.tile_pool(name="s", bufs=1))
            ppool = ctx.enter_context(tc.tile_pool(name="p", bufs=1, space="PSUM"))

            # ---- constants (DMAs emitted lazily, spread across early layers
            # so they never queue ahead of a layer's own weight DMAs) ----
            oh_sb = {}
            gidx_sb = {}
            sv_sb = {}

            def load_oh(s, idxs):
                omap = st[f'ohmap{s}']
                for i in idxs:
                    g, c = omap[i]
                    lo, hi = st[f'ohspan{s}'][(g, c)]
                    t = cpool.tile([128, 512], dt.bfloat16, name=f"oh{s}_{i}", bufs=1)
                    nc.sync.dma_start(out=t[:, lo:hi], in_=D[f'oh{s}'][i][:, lo:hi])
                    oh_sb[(s, g, c)] = t

            sel_sb = []

            def load_gidx(s):
                pad = _pad16(len(st['slots'][s]))
                t = cpool.tile([128, pad // 16], dt.int16, name=f"gidx{s}", bufs=1)
                nc.sync.dma_start(out=t[:], in_=D[f'gidx{s}'][:])
                gidx_sb[s] = t

            def load_sel(v0, v1):
                for v in range(v0, min(v1, nv)):
                    t = cpool.tile([128, NS4c], dt.float32r, name=f"sel{v}", bufs=1)
                    nc.sync.dma_start(out=t[:], in_=D['sel4'][v])
                    sel_sb.append(t)

            def load_sv(s):
                wid = 2 * st['NG4'] if s == 4 else 2
                t = cpool.tile([128, wid], dt.float32, name=f"sv{s}", bufs=1)
                nc.sync.dma_start(out=t[:], in_=D[f'sv{s}'][:])
                sv_sb[s] = t
            midx_s = cpool.tile([128, DELTA // 16], dt.int16, name="midx_s", bufs=1)
            midx_e = cpool.tile([128, DELTA // 16], dt.int16, name="midx_e", bufs=1)
            eps_t = cpool.tile([128, 1], dt.float32, name="eps_t", bufs=1)
            nc.vector.memset(eps_t[:], EPS)
            zero32 = cpool.tile([128, DELTA], dt.float32, name="zero32", bufs=1)
            nc.vector.memset(zero32[:], 0.0)
            identb = cpool.tile([128, 128], dt.bfloat16, name="identb", bufs=1)
            nc.sync.dma_start(out=identb[:], in_=D['identb'][:])
            identity = cpool.tile([128, 128], dt.float32r, name="identity", bufs=1)
            nc.sync.dma_start(out=identity[:], in_=D['ident'][:])
            zsq_f = cpool.tile([128, 128], dt.float32, name="zsq_f", bufs=1)
            nc.vector.memset(zsq_f[:], 0.0)
            zsq_r = cpool.tile([128, 128], dt.float32r, name="zsq_r", bufs=1)
            nc.scalar.activation(out=zsq_r[:], in_=zsq_f[:], func=AF.Copy)
            # zero sources for psum-bank "start" dummies: one start=True matmul
            # (or transpose) marks the whole 2KB zero-region; real matmuls then
            # run with start=False so disjoint column ranges don't wipe each
            # other's accumulation.
            zrow_b = cpool.tile([1, 640], dt.bfloat16, name="zrow_b", bufs=1)
            nc.vector.memset(zrow_b[:], 0.0)
            zsq_b = cpool.tile([128, 128], dt.bfloat16, name="zsq_b", bufs=1)
            nc.vector.memset(zsq_b[:], 0.0)
            gidx_dum = cpool.tile([128, 1], dt.int16, name="gidx_dum", bufs=1)
            nc.vector.memset(gidx_dum[:], 0)

            f0t = fpool.tile([128, W_COLS], dt.float32r, name="f0t", tag="f", bufs=8)
            nc.sync.dma_start(out=f0t[:1, :], in_=D['f0'][:])
            fin = [f0t]

            for li in range(10):
                s = li // 2
                Ci, Co = CIN[li], _co_dev(li)
                cin_t = (Ci + 127) // 128
                cout_t = (Co + 127) // 128
                n_bn = st['n_act'][s]
                slots = st['slots'][s] if s >= 2 else []
                nslots = len(slots)
                has_pairs = nslots > 0
                NSp = _pad16(nslots)
                ngroups = (nslots + 127) // 128 if has_pairs else 0

                # center weights + bn params first so their DMAs lead the queue
                wc_sb = []
                for k in range(cin_t):
                    kk = min(128, Ci - k * 128)
                    t = wpool.tile([128, Co], dt.float32r, name=f"wc{li}_{k}", tag="wc", bufs=6)
                    nc.sync.dma_start(out=t[:kk, :], in_=D[f'wc{li}'][k * 128:k * 128 + kk, :])
                    wc_sb.append((t, kk))
                gbt = spool.tile([128, cout_t * 2], dt.float32, name=f"gbt{li}", tag="gbt", bufs=2)
                nc.sync.dma_start(out=gbt[:].rearrange("p (m two) -> p m two", m=cout_t),
                                  in_=D[f'gb{li}'].rearrange("m p two -> p m two"))

                nk4 = len(st['ohmap4'])
                if li == 1:
                    load_gidx(2); load_sv(2); load_oh(2, range(len(st['ohmap2'])))
                    load_gidx(3); load_sv(3); load_oh(3, range(len(st['ohmap3'])))
                elif li == 2:
                    load_sv(4)
                    nc.sync.dma_start(out=midx_s[:], in_=D['midx_s'][:])
                    nc.sync.dma_start(out=midx_e[:], in_=D['midx_e'][:])
                elif li == 3:
                    load_sel(0, 3)
                elif li == 4:
                    load_sel(3, nv); load_oh(4, range(0, nk4 // 3))
                elif li == 5:
                    load_oh(4, range(nk4 // 3, 2 * nk4 // 3))
                elif li == 6:
                    load_oh(4, range(2 * nk4 // 3, nk4))

                Gbf, yt_sb = [], []
                P8 = s >= 2 and FP8S[s]
                pdt = dt.float8e4 if P8 else dt.bfloat16
                T = (cin_t + 1) // 2 if P8 else cin_t
                if has_pairs:
                    if li == 8:
                        # merge deltas into fin[k][:, 0:DELTA]
                        for k in range(cin_t):
                            sg = spool.tile([128, DELTA], dt.float32, name=f"sg{li}_{k}", tag="mrg", bufs=4)
                            eg = spool.tile([128, DELTA], dt.float32, name=f"eg{li}_{k}", tag="mrg", bufs=4)
                            fk32 = fin[k][:].bitcast(dt.float32)
                            nc.gpsimd.ap_gather(out_ap=sg[:], in_ap=fk32, idxs_ap=midx_s[:],
                                                channels=128, num_elems=W_COLS, d=1, num_idxs=DELTA)
                            nc.gpsimd.ap_gather(out_ap=eg[:], in_ap=fk32, idxs_ap=midx_e[:],
                                                channels=128, num_elems=W_COLS, d=1, num_idxs=DELTA)
                            nc.vector.tensor_tensor(out=eg[:], in0=eg[:], in1=sg[:], op=ALU.subtract)
                            nc.vector.tensor_scalar_max(out=eg[:], in0=eg[:], scalar1=0.0)
                            nc.scalar.activation(out=fin[k][:, 0:DELTA], in_=eg[:], func=AF.Copy)
                    # G tiles are [128, ktc, N] (ktc k-tiles packed on the free
                    # axis) so fp8 DoubleRow can contract two k-tiles per pass.
                    def ktc_of(t):
                        return min(2, cin_t - 2 * t) if P8 else 1

                    def k_of(t, i):
                        return 2 * t + i if P8 else t

                    if s < 4:
                        for t_ in range(T):
                            ktc = ktc_of(t_)
                            gd = spool.tile([128, ktc, NSp], pdt, name=f"gd{li}_{t_}", tag="gbf", bufs=5)
                            for i in range(ktc):
                                g32 = spool.tile([128, NSp], dt.float32, name=f"g32_{li}_{t_}_{i}", tag="g32", bufs=2)
                                nc.gpsimd.ap_gather(out_ap=g32[:], in_ap=fin[k_of(t_, i)][:].bitcast(dt.float32),
                                                    idxs_ap=gidx_sb[s][:], channels=128,
                                                    num_elems=W_COLS, d=1, num_idxs=NSp)
                                nc.vector.tensor_copy(out=gd[:, i, :], in_=g32[:])
                            Gbf.append(gd)
                    else:
                        # G via PE transpose + selection matmuls (NOT gpsimd
                        # gathers: blocked gpsimd semaphore waits wake up with
                        # ~11-14us latency, which serializes the whole layer)
                        ft_sb = []
                        for v in range(nv):
                            ftp = ppool.tile([128, 512], dt.float32r, name=f"ftp{li}_{v}", tag="bank", bufs=8)
                            nc.tensor.matmul(ftp[:, 0:16], lhsT=zsq_r[:], rhs=identity[:, :16],
                                             is_transpose=True, start=True, stop=False)
                            for k in range(cin_t):
                                nc.tensor.matmul(ftp[:, k * 128:(k + 1) * 128],
                                                 lhsT=fin[k][:, v * 128:(v + 1) * 128],
                                                 rhs=identity[:], is_transpose=True,
                                                 start=False, stop=(k == cin_t - 1))
                            t = spool.tile([128, cin_t * 128], dt.float32r, name=f"ft{li}_{v}", tag="ft", bufs=9)
                            nc.vector.tensor_copy(out=t[:], in_=ftp[:, :cin_t * 128])
                            ft_sb.append(t)
                        nsc = (NS4c + 511) // 512
                        for k in range(cin_t):
                            gb_ = spool.tile([128, NS4c], dt.bfloat16, name=f"gbf{li}_{k}", tag="gbf", bufs=5)
                            Gbf.append(gb_)
                            for c in range(nsc):
                                w_ = min(512, NS4c - c * 512)
                                gp = ppool.tile([128, 512], dt.float32, name=f"gp{li}_{k}_{c}", tag="bank", bufs=8)
                                for v in range(nv):
                                    nc.tensor.matmul(gp[:, :w_], lhsT=ft_sb[v][:, k * 128:(k + 1) * 128],
                                                     rhs=sel_sb[v][:, c * 512:c * 512 + w_],
                                                     start=(v == 0), stop=(v == nv - 1))
                                nc.vector.tensor_copy(out=gb_[:, c * 512:c * 512 + w_], in_=gp[:, :w_])

                    if s == 4:
                        # transposed pair GEMM: YT[slot, cout] directly, per
                        # wid-block (32-padded starts).  Groups processed in
                        # waves of <=5 psum banks; weights DMA'd per wave.
                        yt_sb = [None] * NG
                        for wg in wave_groups:
                            if not wg:
                                continue
                            blks = [b for b in blocks4 if b[3] // 128 in wg]
                            wlo = min(b[0] for b in blks)
                            whi = max(b[0] for b in blks) + 1
                            ytp = {}
                            for g in wg:
                                t = ppool.tile([128, 512], dt.float32, name=f"ytp{li}_{g}", tag="bank", bufs=8)
                                nc.tensor.matmul(t[:, :16], lhsT=zrow_b[:1, :128],
                                                 rhs=zrow_b[:1, 128:144], start=True, stop=False)
                                ytp[g] = t
                            wmid = (wlo + whi + 1) // 2
                            for k in range(cin_t):
                                for h, (h0, h1) in enumerate(((wlo, wmid), (wmid, whi))):
                                    if h1 <= h0:
                                        continue
                                    wpt = wpool.tile([128, h1 - h0, Co], dt.bfloat16,
                                                     name=f"wpt{li}_{k}_{wg[0]}_{h}", tag=f"wpt{h}", bufs=2)
                                    nc.sync.dma_start(out=wpt[:],
                                                      in_=D[f'wp{li}'][k, :, h0 * Co:h1 * Co].rearrange(
                                                          "p (w c) -> p w c", w=h1 - h0))
                                    for (w, clo, chi, plo) in blks:
                                        if not (h0 <= w < h1):
                                            continue
                                        g = plo // 128
                                        rel = plo - g * 128
                                        nc.tensor.matmul(
                                            ytp[g][rel:rel + (chi - clo), :Co],
                                            lhsT=Gbf[k][:, clo:chi],
                                            rhs=wpt[:, w - h0],
                                            start=False, stop=(k == cin_t - 1))
                            for g in wg:
                                t = spool.tile([128, Co], dt.bfloat16, name=f"yt{li}_{g}", tag="yt", bufs=10)
                                nc.scalar.activation(out=t[:], in_=ytp[g][:, :Co], func=AF.Copy,
                                                     scale=sv_sb[4][:, (li - 8) * NG + g:(li - 8) * NG + g + 1])
                                yt_sb[g] = t
                        ngroups = NG
                    else:
                        # small stages: non-transposed Y then PE transpose
                        ranges = slot_ranges(slots)
                        nsc = (NSp + 511) // 512
                        yp = [[None] * nsc for _ in range(cout_t)]
                        for m in range(cout_t):
                            for c in range(nsc):
                                yp[m][c] = ppool.tile([128, 512], dt.float32, name=f"yp{li}_{m}_{c}", tag="bank", bufs=8)
                                nc.tensor.matmul(yp[m][c][:, :16], lhsT=zrow_b[:1, :128],
                                                 rhs=zrow_b[:1, 128:144], start=True, stop=False)
                        nwid = len(st['act_off'][s])
                        nw0 = (nwid + 1) // 2
                        for t_ in range(T):
                            ktc = ktc_of(t_)
                            for h, (w0, w1) in enumerate(((0, nw0), (nw0, nwid))):
                                nh = w1 - w0
                                if nh <= 0:
                                    continue
                                wpt = wpool.tile([128, nh, ktc, Co], pdt, name=f"wpt{li}_{t_}_{h}", tag=f"wpt{h}", bufs=2)
                                for i in range(ktc):
                                    nc.sync.dma_start(out=wpt[:, :, i, :],
                                                      in_=D[f'wp{li}'][(2 * t_ if P8 else t_) + i, :,
                                                                       w0 * Co:w1 * Co].rearrange(
                                                          "p (w c) -> p w c", w=nh))
                                for (w, lo, hi) in ranges:
                                    if not (w0 <= w < w1):
                                        continue
                                    c = lo // 512
                                    for m in range(cout_t):
                                        pm = min(128, Co - m * 128)
                                        last = t_ == T - 1
                                        if P8 and ktc == 2:
                                            nc.tensor.matmul(
                                                yp[m][c][:pm, lo - c * 512:hi - c * 512],
                                                lhsT=wpt[:, w - w0, :, m * 128:m * 128 + pm],
                                                rhs=Gbf[t_][:, :, lo:hi],
                                                perf_mode=mybir.MatmulPerfMode.DoubleRow,
                                                start=False, stop=last)
                                        else:
                                            for i in range(ktc):
                                                nc.tensor.matmul(
                                                    yp[m][c][:pm, lo - c * 512:hi - c * 512],
                                                    lhsT=wpt[:, w - w0, i, m * 128:m * 128 + pm],
                                                    rhs=Gbf[t_][:, i, lo:hi],
                                                    start=False, stop=last and i == ktc - 1)
                        NSG = ngroups * 128
                        ysb = []
                        for m in range(cout_t):
                            t = spool.tile([128, NSG], dt.bfloat16, name=f"ysb{li}_{m}", tag="ysb", bufs=5)
                            if NSG > NSp:
                                nc.vector.memset(t[:, NSp:], 0.0)
                            for c in range(nsc):
                                w_ = min(512, NSp - c * 512)
                                nc.vector.tensor_copy(out=t[:, c * 512:c * 512 + w_], in_=yp[m][c][:, :w_])
                            ysb.append(t)
                        for g in range(ngroups):
                            ytp = ppool.tile([128, 512], dt.bfloat16, name=f"ytp{li}_{g}", tag="bank", bufs=8)
                            nc.tensor.matmul(ytp[:, 0:16], lhsT=zsq_b[:], rhs=identb[:, :16],
                                             is_transpose=True, start=True, stop=False)
                            for m in range(cout_t):
                                nc.tensor.matmul(ytp[:, m * 128:(m + 1) * 128],
                                                 lhsT=ysb[m][:, g * 128:(g + 1) * 128],
                                                 rhs=identb[:], is_transpose=True,
                                                 start=False, stop=(m == cout_t - 1))
                            t = spool.tile([128, Co], dt.bfloat16, name=f"yt{li}_{g}", tag="yt", bufs=10)
                            nc.scalar.activation(out=t[:], in_=ytp[:, :Co], func=AF.Copy,
                                                 scale=sv_sb[s][:, li - 2 * s:li - 2 * s + 1])
                            yt_sb.append(t)

                # ---- center GEMM + scatter + BN ----
                fout = []
                for m in range(cout_t):
                    pm = min(128, Co - m * 128)
                    fo = fpool.tile([128, W_COLS], dt.float32r, name=f"f{li}_{m}", tag="f", bufs=8)
                    fout.append(fo)
                    ps_c = []
                    for c in range(4):
                        ps = ppool.tile([128, 512], dt.float32, name=f"ps{li}_{m}_{c}", tag="bank", bufs=8)
                        ps_c.append(ps)
                        mms = [('c', k) for k in range(cin_t)]
                        if has_pairs:
                            mms += [('s', g) for g in range(ngroups) if (s, g, c) in oh_sb]
                        for i, (kind, a) in enumerate(mms):
                            last = i == len(mms) - 1
                            if kind == 'c':
                                wt, kk = wc_sb[a]
                                nc.tensor.matmul(
                                    ps[:pm, :], lhsT=wt[:kk, m * 128:m * 128 + pm],
                                    rhs=fin[a][:kk, DELTA + c * 512:DELTA + (c + 1) * 512],
                                    start=(a == 0), stop=last)
                            else:
                                lo, hi = st[f'ohspan{s}'][(a, c)]
                                nc.tensor.matmul(
                                    ps[:pm, lo:hi], lhsT=yt_sb[a][:, m * 128:m * 128 + pm],
                                    rhs=oh_sb[(s, a, c)][:, lo:hi], start=False, stop=last)
                    stats = spool.tile([128, 4, 6], dt.float32, name=f"bs{li}_{m}", tag="bs", bufs=3)
                    for c in range(4):
                        hi = min(512, n_bn - c * 512)
                        nc.vector.bn_stats(out=stats[:pm, c, :], in_=ps_c[c][:pm, :hi])
                    mv = spool.tile([128, 2], dt.float32, name=f"mv{li}_{m}", tag="mv", bufs=3)
                    nc.vector.bn_aggr(out=mv[:pm, :], in_=stats[:pm, :, :])
                    scale = spool.tile([128, 1], dt.float32, name=f"sc{li}_{m}", tag="sc", bufs=3)
                    shift = spool.tile([128, 1], dt.float32, name=f"sh{li}_{m}", tag="sh", bufs=3)
                    tmp = spool.tile([128, 1], dt.float32, name=f"tp{li}_{m}", tag="tp", bufs=3)
                    nc.scalar.activation(out=scale[:pm, :], in_=mv[:pm, 1:2], func=AF.Abs_reciprocal_sqrt,
                                         bias=eps_t[:pm, :], scale=1.0)
                    nc.vector.tensor_tensor(out=scale[:pm, :], in0=scale[:pm, :],
                                            in1=gbt[:pm, 2 * m:2 * m + 1], op=ALU.mult)
                    nc.vector.tensor_tensor(out=tmp[:pm, :], in0=mv[:pm, 0:1],
                                            in1=scale[:pm, :], op=ALU.mult)
                    nc.vector.tensor_tensor(out=shift[:pm, :], in0=gbt[:pm, 2 * m + 1:2 * m + 2],
                                            in1=tmp[:pm, :], op=ALU.subtract)
                    nc.scalar.activation(out=fo[:, 0:DELTA], in_=zero32[:], func=AF.Copy)
                    for c in range(4):
                        nc.scalar.activation(out=fo[:pm, DELTA + c * 512:DELTA + (c + 1) * 512],
                                             in_=ps_c[c][:pm, :], func=AF.Relu,
                                             bias=shift[:pm, :], scale=scale[:pm, :])
                        if li == 9:
                            nc.sync.dma_start(out=D['out'][0][:, c * 512:(c + 1) * 512],
                                              in_=fo[:, DELTA + c * 512:DELTA + (c + 1) * 512])
                fin = fout
    nc.compile()
    return nc


# --------------------------------------------------------------------------
# entry point
# --------------------------------------------------------------------------

_CACHE = {}


def _get(coors):
    key = coors.tobytes()
    if key not in _CACHE:
        st = build_structure(coors)
        nc = build_program(st)
        _CACHE[key] = (st, nc)
    return _CACHE[key]


def run_device(st, nc, inputs, trace=False):
    from concourse.bass_utils import run_bass_kernel_spmd
    arrs = pack_structure(st, inputs)
    base = {k: np.ascontiguousarray(v) for k, v in arrs.items()}
    # li9 is cout-sharded: core c computes output channels of chunk c//2
    in_maps = []
    nwid9 = base['wp9'].shape[2] // COUT[9]
    wp9v = base['wp9'].reshape(base['wp9'].shape[0], 128, nwid9, COUT[9])
    for c in range(8):
        m = c // 2
        d = dict(base)
        d['wc9'] = np.ascontiguousarray(base['wc9'][:, m * 128:(m + 1) * 128])
        d['wp9'] = np.ascontiguousarray(
            wp9v[:, :, :, m * 128:(m + 1) * 128]).reshape(wp9v.shape[0], 128, nwid9 * 128)
        d['gb9'] = np.ascontiguousarray(base['gb9'][m:m + 1])
        in_maps.append(d)
    res = run_bass_kernel_spmd(nc, in_maps, list(range(8)), trace=trace)
    return res


def gather_f9(res):
    # rows m*128:(m+1)*128 of f9 live on cores 2m and 2m+1
    return np.concatenate([res.results[2 * m]['out'].reshape(128, N_VOX)
                           for m in range(4)], axis=0)


def postprocess(st, f9):
    prep = st['prep']
    col = st['col'] - DELTA
    out_vals = {}
    for members in prep['pool_groups'][4]:
        surv = members[0]
        val = f9[:, col[surv]].copy()
        for e in members[1:]:
            val = np.maximum(val, f9[:, col[e]])
        out_vals[surv] = val
    D5, H5, W5 = DIMS[5]
    dense = np.zeros((BATCH, D5, H5, W5, 512), np.float32)
    fa = prep['final_active']; fc = prep['final_coords']; fb = prep['final_bidx']
    for v in np.where(fa)[0]:
        val = out_vals.get(int(v))
        if val is None:
            val = f9[:, col[v]]
        dense[fb[v], fc[v][0], fc[v][1], fc[v][2]] += val
    return dense.transpose(0, 4, 1, 2, 3).reshape(BATCH, 512 * D5, H5, W5)


def kernel(**inputs):
    coors = np.asarray(inputs['coors'])
    st, nc = _get(coors)
    res = run_device(st, nc, inputs)
    return postprocess(st, gather_f9(res))

